# revision 1
# baseline (speedup 1.0000x reference)
"""nn_DiTBlock on 8 TRN2 NeuronCores: data-parallel over batch (B=8), one
batch element per core. Self-contained: builds the Bass/Tile kernel, shards
inputs on the host (transpose/pack/cast only), runs SPMD via bass2jax/PJRT,
gathers and un-transposes the output.

v2 design (vs baseline): all GEMM operands bf16 (psum stays f32), weight DMAs
merged via 3D access patterns, softmax ones-column moved to column 0 so the
denominator lands on psum partition 0 (partition_broadcast source) without an
SBUF round-trip, attention output assembled by SBUF->SBUF DMA instead of a
DRAM round-trip, Rsqrt/pow fusions replace Sqrt+reciprocal chains, residual
updates fused into single scalar_tensor_tensor ops with gate*bias constants
pre-added to the residual, and per-head-pair qk-generation is interleaved
with attention so Exp activations overlap the next pair's matmuls."""

import numpy as np
from contextlib import ExitStack

import concourse.bass as bass
import concourse.mybir as mybir
import concourse.tile as tile
from concourse import bacc

F32 = mybir.dt.float32
F32R = mybir.dt.float32r
BF16 = mybir.dt.bfloat16
AF = mybir.ActivationFunctionType
OP = mybir.AluOpType

B, T, D, H = 8, 1024, 1024, 16
HD = D // H          # 64
DM = 4 * D           # 4096
NCH = D // 128       # 8
P = 128


def host_prep(x, c, g1, g2, gq, gk, Wqkv, bqkv, Wproj, bproj,
              Wfc1, bfc1, Wfc2, bfc2, Wada, bada):
    import ml_dtypes
    bf16 = ml_dtypes.bfloat16

    def packT(W):  # (F, K) -> (K//128, 128, F) contiguous, bf16
        Wt = np.ascontiguousarray(np.asarray(W).T).astype(bf16)
        K, F = Wt.shape
        return np.ascontiguousarray(Wt.reshape(K // 128, 128, F))

    f32 = np.float32
    com = {
        "wqkv": packT(Wqkv), "wproj": packT(Wproj),
        "wfc1": packT(Wfc1), "wfc2": packT(Wfc2), "wada": packT(Wada),
        "bqkv": np.asarray(bqkv, f32), "bproj": np.asarray(bproj, f32),
        "bfc1": np.asarray(bfc1, f32), "bfc2": np.asarray(bfc2, f32),
        "bada": np.asarray(bada, f32),
        "g": np.stack([np.asarray(g1)[0], np.asarray(g2)[0],
                       np.asarray(gq)[0], np.asarray(gk)[0]]).astype(f32),
    }
    in_maps = []
    for b in range(B):
        m = dict(com)
        m["xt"] = np.ascontiguousarray(np.asarray(x[b], f32).T)
        m["cvec"] = np.asarray(c[b], f32)
        in_maps.append(m)
    return in_maps


def host_post(results):
    return np.ascontiguousarray(
        np.stack([r["out"].T for r in results]).astype(np.float32))


def col_ap(handle, nch):
    """DRAM (nch*128,) viewed as [128, nch]: tile[p, ch] = v[ch*128+p]."""
    return bass.AP(tensor=handle, offset=0, ap=[[1, P], [P, nch]])


def bc_ap(handle, n, offset=0):
    """DRAM (n,) broadcast-read to [128, n] (partition stride 0)."""
    return bass.AP(tensor=handle, offset=offset, ap=[[0, P], [1, n]])


def wslab_ap(handle, F, c0, ncols, nch=NCH, doff=0):
    """DRAM weight (nch_tot, 128, F) -> SBUF [128, nch, ncols] in one DMA:
    tile[p, d, c] = w[doff+d, p, c0+c]."""
    return bass.AP(tensor=handle, offset=doff * P * F + c0,
                   ap=[[F, P], [P * F, nch], [1, ncols]])


def _pin_exp_ln_table():
    """Make every Exp/Ln activation resolve to the one act-func table that
    holds both ('natural_log_exp_and_others'), so the rsqrt-via-exp(ln) in
    the attention loop never forces a LoadActFuncSet between head pairs.
    Indices into act_info.json are preserved; only the chooser's view of
    which sets contain exp/ln is narrowed."""
    import functools
    import concourse.hw_specs as hws
    import concourse.bacc as bacc_mod
    if getattr(hws, "_exp_ln_pinned", False):
        return
    orig = hws.get_activation_tables

    @functools.cache
    def patched(module_arch):
        tabs = {k: set(v) for k, v in orig(module_arch).items()}
        keep = "natural_log_exp_and_others"
        if keep in tabs:
            for name, s in tabs.items():
                if name != keep:
                    s.discard(AF.Exp)
                    s.discard(AF.Ln)
        return tabs

    hws.get_activation_tables = patched
    bacc_mod.get_activation_tables = patched
    hws._exp_ln_pinned = True


def build_dit(n_cores=8, repeat=1):
    _pin_exp_ln_table()
    nc = bacc.Bacc("TRN2", target_bir_lowering=False, debug=False,
                   num_devices=n_cores)

    xt = nc.dram_tensor("xt", [D, T], F32, kind="ExternalInput")
    cin = nc.dram_tensor("cvec", [D], F32, kind="ExternalInput")
    g = nc.dram_tensor("g", [4], F32, kind="ExternalInput")
    wqkv = nc.dram_tensor("wqkv", [NCH, P, 3 * D], BF16, kind="ExternalInput")
    wproj = nc.dram_tensor("wproj", [NCH, P, D], BF16, kind="ExternalInput")
    wfc1 = nc.dram_tensor("wfc1", [NCH, P, DM], BF16, kind="ExternalInput")
    wfc2 = nc.dram_tensor("wfc2", [DM // P, P, D], BF16, kind="ExternalInput")
    wada = nc.dram_tensor("wada", [NCH, P, 6 * D], BF16, kind="ExternalInput")
    bqkv = nc.dram_tensor("bqkv", [3 * D], F32, kind="ExternalInput")
    bproj = nc.dram_tensor("bproj", [D], F32, kind="ExternalInput")
    bfc1 = nc.dram_tensor("bfc1", [DM], F32, kind="ExternalInput")
    bfc2 = nc.dram_tensor("bfc2", [D], F32, kind="ExternalInput")
    bada = nc.dram_tensor("bada", [6 * D], F32, kind="ExternalInput")
    out = nc.dram_tensor("out", [D, T], F32, kind="ExternalOutput")

    with tile.TileContext(nc, pool_alloc_mode="queue") as tc:
      for _rep in range(repeat):
        with ExitStack() as X:
            const = X.enter_context(tc.tile_pool(name="const", bufs=1))
            resid = X.enter_context(tc.tile_pool(name="resid", bufs=1))
            dram = X.enter_context(tc.tile_pool(name="dram", bufs=1, space="DRAM"))

            # ---------------- input DMAs (SP queue order = priority) -------
            c_pm = const.tile([P, NCH], F32)
            nc.sync.dma_start(out=c_pm, in_=col_ap(cin, NCH))
            g_bc = const.tile([P, 4], F32)
            nc.sync.dma_start(out=g_bc, in_=bc_ap(g, 4))

            adas = ExitStack()
            wadap = adas.enter_context(tc.tile_pool(name="wadap", bufs=1))
            wada_sb = wadap.tile([P, NCH, 6 * D], BF16)
            for q4 in range(4):
                nc.sync.dma_start(
                    out=wada_sb[:, :, q4 * 1536:(q4 + 1) * 1536],
                    in_=wslab_ap(wada, 6 * D, q4 * 1536, 1536))

            x_res = resid.tile([P, NCH, T], F32)
            nc.sync.dma_start(out=x_res, in_=bass.AP(
                tensor=xt, offset=0, ap=[[T, P], [P * T, NCH], [1, T]]))
            badaT = const.tile([P, 48], F32)
            nc.sync.dma_start(out=badaT, in_=col_ap(bada, 48))

            bqkv_c = const.tile([P, 3 * D // P], F32)
            nc.sync.dma_start(out=bqkv_c, in_=col_ap(bqkv, 3 * D // P))
            bproj_c = const.tile([P, NCH], F32)
            nc.sync.dma_start(out=bproj_c, in_=col_ap(bproj, NCH))
            bfc1_c = const.tile([P, DM // P], F32)
            nc.sync.dma_start(out=bfc1_c, in_=col_ap(bfc1, DM // P))
            bfc2_c = const.tile([P, NCH], F32)
            nc.sync.dma_start(out=bfc2_c, in_=col_ap(bfc2, NCH))
            vbias_bc = const.tile([P, D], BF16)
            nc.gpsimd.dma_start(out=vbias_bc, in_=bc_ap(bqkv, D, offset=2 * D))

            # ---------------- constants ----------------
            gsq = const.tile([P, 4], F32)
            nc.vector.tensor_tensor(gsq, g_bc, g_bc, OP.mult)
            ginv2 = const.tile([P, 4], F32)
            nc.vector.reciprocal(ginv2, gsq)
            scl_n1 = const.tile([P, 1], F32)
            nc.vector.tensor_scalar_mul(scl_n1, ginv2[:, 0:1], 1.0 / D)
            scl_n2 = const.tile([P, 1], F32)
            nc.vector.tensor_scalar_mul(scl_n2, ginv2[:, 1:2], 1.0 / D)
            scl_q = const.tile([P, 1], F32)
            nc.vector.tensor_copy(scl_q, ginv2[:, 2:3])
            scl_k = const.tile([P, 1], F32)
            nc.vector.tensor_scalar_mul(scl_k, ginv2[:, 3:4], 1.0 / HD)

            ones1 = const.tile([P, 1], BF16)
            nc.gpsimd.memset(ones1, 1.0)
            onesh = const.tile([P, 2], BF16)
            nc.gpsimd.memset(onesh, 0.0)
            nc.gpsimd.memset(onesh[0:64, 0:1], 1.0)
            nc.gpsimd.memset(onesh[64:128, 1:2], 1.0)

            cs_pm = const.tile([P, NCH], BF16)
            nc.scalar.activation(cs_pm, c_pm, AF.Silu)

            # ---------------- adaLN ----------------
            # cols: shift_msa 0:8 | scale_msa 8:16 | gate_msa 16:24
            #       shift_mlp 24:32 | scale_mlp 32:40 | gate_mlp 40:48
            ada_row = wadap.tile([1, 6 * D], F32, tag="arow")
            ada_scr = dram.tile([1, 6 * D], F32)
            adaT = const.tile([P, 48], F32)
            with tc.tile_pool(name="psA", bufs=2, space="PSUM") as psA:
                for nb in range(6 * D // 512):
                    pa = psA.tile([1, 512], F32, name="pa")
                    for d in range(NCH):
                        nc.tensor.matmul(pa, cs_pm[:, d:d + 1],
                                         wada_sb[:, d, nb * 512:(nb + 1) * 512],
                                         start=(d == 0), stop=(d == NCH - 1))
                    nc.scalar.activation(ada_row[:, nb * 512:(nb + 1) * 512],
                                         pa, AF.Identity)
            nc.sync.dma_start(out=ada_scr, in_=ada_row)
            nc.sync.dma_start(out=adaT, in_=bass.AP(
                tensor=ada_scr.tensor, offset=0, ap=[[1, P], [P, 48]]))
            nc.vector.tensor_tensor(adaT, adaT, badaT, OP.add)
            nc.vector.tensor_scalar_add(adaT[:, 8:16], adaT[:, 8:16], 1.0)
            nc.vector.tensor_scalar_add(adaT[:, 32:40], adaT[:, 32:40], 1.0)
            gb_proj = const.tile([P, NCH], F32)
            nc.vector.tensor_tensor(gb_proj, adaT[:, 16:24], bproj_c, OP.mult)
            gb_fc2 = const.tile([P, NCH], F32)
            nc.vector.tensor_tensor(gb_fc2, adaT[:, 40:48], bfc2_c, OP.mult)

            def norm_sums(src, scl, nrm, sqp, psN):
                """rmsnorm 1/||.|| factor of src, broadcast to [P, T]."""
                pss = [psN.tile([1, 512], F32, name="pss") for _ in range(2)]
                for j in range(NCH):
                    xsq = sqp.tile([P, T], BF16, name="xsq")
                    nc.scalar.activation(xsq, src[:, j, :], AF.Square)
                    for t2 in range(2):
                        nc.tensor.matmul(pss[t2], ones1,
                                         xsq[:, t2 * 512:(t2 + 1) * 512],
                                         start=(j == 0), stop=(j == NCH - 1))
                rr = nrm.tile([1, T], F32, name="rr", tag="rr")
                rinv = nrm.tile([1, T], BF16, name="rinv", tag="ri")
                for t2 in range(2):
                    ts2 = slice(t2 * 512, (t2 + 1) * 512)
                    nc.scalar.activation(rr[:, ts2], pss[t2], AF.Ln,
                                         scale=scl[0:1, :])
                    nc.scalar.activation(rinv[:, ts2], rr[:, ts2], AF.Exp,
                                         scale=-0.5)
                rbc = nrm.tile([P, T], BF16, name="rbc", tag="rb")
                for t2 in range(2):
                    ts2 = slice(t2 * 512, (t2 + 1) * 512)
                    nc.gpsimd.partition_broadcast(rbc[:, ts2], rinv[:, ts2])
                return rbc

            def mod_apply(src, rbc, sh_col, sc_col, h_out, ts, xnp):
                n = ts.stop - ts.start
                for j in range(NCH):
                    xn = xnp.tile([P, n], BF16, name="xn")
                    nc.vector.tensor_tensor(xn, src[:, j, ts], rbc[:, ts],
                                            OP.mult)
                    nc.vector.tensor_scalar(h_out[:, j, ts], xn,
                                            adaT[:, sc_col + j:sc_col + j + 1],
                                            adaT[:, sh_col + j:sh_col + j + 1],
                                            OP.mult, OP.add)

            def norm_modulate(src, scl, sh_col, sc_col, h_out):
                with tc.tile_pool(name="sqp", bufs=3) as sqp, \
                     tc.tile_pool(name="psN", bufs=2, space="PSUM") as psN, \
                     tc.tile_pool(name="nrm", bufs=1) as nrm, \
                     tc.tile_pool(name="xnp", bufs=3) as xnp:
                    rbc = norm_sums(src, scl, nrm, sqp, psN)
                    mod_apply(src, rbc, sh_col, sc_col, h_out,
                              slice(0, T), xnp)

            att = ExitStack()

            h1p = att.enter_context(tc.tile_pool(name="h1p", bufs=1, side="right"))
            h1 = h1p.tile([P, NCH, T], BF16)
            vxp = att.enter_context(tc.tile_pool(name="vxp", bufs=1, side="right"))
            vx = vxp.tile([P, NCH, H, HD + 1], BF16)  # [kt_chunk][head][(v,1)]
            nc.gpsimd.memset(vx[:, :, :, HD:HD + 1], 1.0)

            # ------------ norm1 + modulate ------------
            norm_modulate(x_res, scl_n1, 0, 8, h1)
            adas.close()

            # ---- gate_msa * bproj pre-added to residual (on Pool) ----
            for j in range(NCH):
                nc.gpsimd.tensor_scalar_add(x_res[:, j, :], x_res[:, j, :],
                                            gb_proj[:, j:j + 1])

            # ------------ interleaved qk-gen + attention per head pair ----
            mlpw = ExitStack()
            wf1p = mlpw.enter_context(tc.tile_pool(name="wf1p", bufs=2))

            def load_wf1(q8):
                t = wf1p.tile([P, NCH, 512], BF16, name="wf1")
                nc.sync.dma_start(out=t, in_=wslab_ap(wfc1, DM, q8 * 512, 512))
                return t

            wpjp = ExitStack()
            wpj = wpjp.enter_context(
                tc.tile_pool(name="wpjp", bufs=1)) \
                .tile([P, NCH, D], BF16)
            oT_sb = wpjp.enter_context(tc.tile_pool(name="otp", bufs=1)) \
                .tile([P, NCH, T], BF16)

            qp_ = att.enter_context(tc.tile_pool(name="qp_", bufs=1))
            kp_ = att.enter_context(tc.tile_pool(name="kp_", bufs=1))
            q_t = qp_.tile([P, NCH, T], BF16)
            k_t = kp_.tile([P, NCH, T], BF16)

            with tc.tile_pool(name="wqp", bufs=4) as wqp, \
                 tc.tile_pool(name="sqq", bufs=2) as sqq, \
                 tc.tile_pool(name="psDR", bufs=2, space="PSUM") as psDR, \
                 tc.tile_pool(name="nrq", bufs=2) as nrq, \
                 tc.tile_pool(name="pscp", bufs=3, space="PSUM") as pscp, \
                 tc.tile_pool(name="psO", bufs=2, space="PSUM") as psO, \
                 tc.tile_pool(name="esp", bufs=4) as esp, \
                 tc.tile_pool(name="onp", bufs=2) as onp:

                # v weights + pair-0 slabs + wproj prefetch
                wqkv_v = wqp.tile([P, NCH, D], BF16, name="wv", tag="wv",
                                  bufs=1)
                nc.sync.dma_start(out=wqkv_v,
                                  in_=wslab_ap(wqkv, 3 * D, 2 * D, D))
                wts = {}
                for fc in (0, 8):
                    wt = wqp.tile([P, NCH, P], BF16, name="wt")
                    nc.sync.dma_start(out=wt, in_=wslab_ap(wqkv, 3 * D, fc * P, P))
                    wts[fc] = wt
                nc.sync.dma_start(out=wpj, in_=wslab_ap(wproj, D, 0, D))

                def vgen(nh):
                    """v for heads nh*8..nh*8+7 (pv tiles share the pscp
                    ring with the score matmuls)."""
                    for t8 in range(NCH):
                        pv = pscp.tile([P, 512], F32, name="pv", tag="psc")
                        for d in range(NCH):
                            nc.tensor.matmul(pv, h1[:, d, t8 * P:(t8 + 1) * P],
                                             wqkv_v[:, d, nh * 512:(nh + 1) * 512],
                                             start=(d == 0), stop=(d == NCH - 1))
                        nc.vector.tensor_tensor(
                            vx[:, t8, nh * 8:(nh + 1) * 8, 0:HD],
                            pv.rearrange("p (h e) -> p h e", e=HD),
                            vbias_bc[:, nh * 512:(nh + 1) * 512].rearrange(
                                "p (h e) -> p h e", e=HD),
                            OP.add)

                krinvs = {}

                def qkgen(jj):
                    # Norm sums in COLUMN form; k's factor rides the Exp
                    # scale, q's is row-ized by a small transpose-DMA.
                    j = jj
                    for fc, tgt, scl in ((j, q_t, scl_q), (8 + j, k_t, scl_k)):
                        isq = fc < 8
                        wt = wts.pop(fc)
                        sq = sqq.tile([P, T], BF16, name="sq")
                        # col layout: q hf-major [hf*8+tc], k tc-major [2tc+hf]
                        pcol = psDR.tile([P, 16], F32, name="pcol", tag="pcol",
                                         bufs=1)
                        for nt in range(2):
                            ts2 = slice(nt * 512, (nt + 1) * 512)
                            psd = psDR.tile([P, 512], F32, name="psd")
                            for d in range(NCH):
                                nc.tensor.matmul(psd, wt[:, d, :],
                                                 h1[:, d, ts2],
                                                 start=(d == 0),
                                                 stop=(d == NCH - 1))
                            nc.vector.tensor_scalar_add(tgt[:, j, ts2], psd,
                                                        bqkv_c[:, fc:fc + 1])
                            nc.vector.tensor_tensor(sq[:, ts2], tgt[:, j, ts2],
                                                    tgt[:, j, ts2], OP.mult)
                            for tc2 in range(4):
                                tci = nt * 4 + tc2
                                if isq:
                                    nc.tensor.matmul(
                                        pcol.rearrange("p (h t) -> p t h",
                                                       h=2)[:, tci, :],
                                        sq[:, tci * P:(tci + 1) * P],
                                        onesh, start=True, stop=True)
                                else:
                                    nc.tensor.matmul(
                                        pcol[:, 2 * tci:2 * tci + 2],
                                        sq[:, tci * P:(tci + 1) * P],
                                        onesh, start=True, stop=True)
                        # prefetch next pair's slabs on the freed buffers
                        if fc == 8 + j and j + 1 < NCH:
                            for nfc in (j + 1, 9 + j):
                                nwt = wqp.tile([P, NCH, P], BF16, name="wt")
                                nc.sync.dma_start(
                                    out=nwt, in_=wslab_ap(wqkv, 3 * D, nfc * P, P))
                                wts[nfc] = nwt
                        # rsqrt via exp(-0.5*ln(x)): stays in the exp/ln
                        # act table (no LoadActFuncSet between pairs)
                        rr = nrq.tile([P, 16], F32, name="rr", tag="rr")
                        nc.scalar.activation(rr, pcol, AF.Ln, scale=scl)
                        rinv_c = nrq.tile([P, 16], F32, name="rinv_c",
                                          tag="ri", bufs=3)
                        nc.scalar.activation(rinv_c, rr, AF.Exp, scale=-0.5)
                        if isq:
                            # row-ize q's factor (p-major permuted row: the
                            # DMA keeps both last dims contiguous) and scale
                            # q_t via a rearranged AP that un-permutes.
                            for hf in range(2):
                                rq_row = nrq.tile([1, T], F32, name="rq_row",
                                                  tag="rq")
                                nc.sync.dma_start(
                                    out=bass.AP(tensor=rq_row.tensor,
                                                offset=rq_row.offset,
                                                ap=[[1, 1], [NCH, P], [1, NCH]]),
                                    in_=rinv_c[:, hf * 8:(hf + 1) * 8])
                                rbc = nrq.tile([P, T], F32, name="rbc",
                                               tag="rb")
                                nc.gpsimd.partition_broadcast(rbc, rq_row)
                                hs = slice(64 * hf, 64 * (hf + 1))
                                nc.vector.tensor_tensor(
                                    tgt[hs, j, :].rearrange(
                                        "p (tc pp) -> p tc pp", tc=NCH),
                                    tgt[hs, j, :].rearrange(
                                        "p (tc pp) -> p tc pp", tc=NCH),
                                    rbc.rearrange(
                                        "p (pp tc) -> p tc pp", tc=NCH)[hs, :, :],
                                    OP.mult)
                        else:
                            krinvs[j] = rinv_c

                def att_head(j, hf):
                    krinv = krinvs[j]
                    h = 2 * j + hf
                    rq = slice(64 * hf, 64 * (hf + 1))
                    if True:
                        for qt in range(2):
                            qs = slice(qt * 512, (qt + 1) * 512)
                            po = psO.tile([65, 512], F32, name="po")
                            for ktc in range(NCH):
                                psc = pscp.tile([P, 512], F32, name="psc")
                                nc.tensor.matmul(psc,
                                                 k_t[rq, j, ktc * P:(ktc + 1) * P],
                                                 q_t[rq, j, qs],
                                                 start=True, stop=True)
                                e = esp.tile([P, 512], BF16, name="es")
                                # k's rmsnorm factor applied inside the Exp
                                nc.scalar.activation(
                                    e, psc, AF.Exp,
                                    scale=krinv[:, 2 * ktc + hf:2 * ktc + hf + 1])
                                nc.tensor.matmul(po, vx[:, ktc, h, :], e,
                                                 start=(ktc == 0),
                                                 stop=(ktc == NCH - 1))
                            rs = onp.tile([65, 512], BF16, name="rs",
                                          tag="rs")
                            with nc.allow_low_precision(reason="softmax denom"):
                                nc.vector.reciprocal(rs[64:65, :], po[64:65, :])
                            rs0 = onp.tile([1, 512], BF16, name="rs0",
                                           tag="rs0")
                            nc.sync.dma_start(out=rs0, in_=rs[64:65, :])
                            rsb = onp.tile([64, 512], BF16, name="rsb", tag="rb")
                            nc.gpsimd.partition_broadcast(rsb, rs0)
                            on = onp.tile([64, 512], BF16, name="on", tag="on")
                            nc.vector.tensor_tensor(on, po[0:64, :],
                                                    rsb, OP.mult)
                            nc.sync.dma_start(out=oT_sb[rq, j, qs], in_=on)


                vgen(0)
                wf1q = []
                qkgen(0)
                for j in range(NCH):
                    if j == 4:
                        vgen(1)
                    elif j == 6:
                        wf1q.append(load_wf1(0))
                    elif j == 7:
                        wf1q.append(load_wf1(1))
                    att_head(j, 0)
                    if j + 1 < NCH:
                        qkgen(j + 1)
                    att_head(j, 1)

            att.close()  # free h1, vx, q_t, k_t (oT_sb stays)

            # ------------ proj + residual ------------
            with tc.tile_pool(name="psP", bufs=6, space="PSUM") as psP:
                for fcb in range(4):
                    ps = [psP.tile([P, 512], F32, name="ps") for _ in range(4)]
                    for d in range(NCH):
                        for fi in range(2):
                            for nt in range(2):
                                nc.tensor.matmul(
                                    ps[2 * fi + nt],
                                    wpj[:, d, fcb * 256 + fi * P:
                                        fcb * 256 + (fi + 1) * P],
                                    oT_sb[:, d, nt * 512:(nt + 1) * 512],
                                    start=(d == 0), stop=(d == NCH - 1))
                    for fi in range(2):
                        fc = fcb * 2 + fi
                        for nt in range(2):
                            nc.vector.scalar_tensor_tensor(
                                x_res[:, fc, nt * 512:(nt + 1) * 512],
                                ps[2 * fi + nt], adaT[:, 16 + fc:17 + fc],
                                x_res[:, fc, nt * 512:(nt + 1) * 512],
                                OP.mult, OP.add)
            wpjp.close()

            # ------------ norm2 + modulate + MLP (token-halved) ------------
            with tc.tile_pool(name="h2p", bufs=1) as h2p, \
                 tc.tile_pool(name="gactp", bufs=1) as gactp, \
                 tc.tile_pool(name="sqp", bufs=3) as sqp2, \
                 tc.tile_pool(name="psN", bufs=2, space="PSUM") as psN2, \
                 tc.tile_pool(name="nrm", bufs=1) as nrm2, \
                 tc.tile_pool(name="xnp", bufs=3) as xnp2, \
                 tc.tile_pool(name="psM", bufs=3, space="PSUM") as psM, \
                 tc.tile_pool(name="psM2", bufs=3, space="PSUM") as psM2, \
                 tc.tile_pool(name="wf2p", bufs=2) as wf2p, \
                 tc.tile_pool(name="evm", bufs=4) as evm:
                h2 = h2p.tile([P, NCH, T], BF16)
                rbc2 = norm_sums(x_res, scl_n2, nrm2, sqp2, psN2)
                for th in range(2):
                    mod_apply(x_res, rbc2, 24, 32, h2,
                              slice(th * 512, (th + 1) * 512), xnp2)
                # gate_mlp * bfc2 pre-added to residual (overlaps fc1)
                for j in range(NCH):
                    nc.gpsimd.tensor_scalar_add(x_res[:, j, :], x_res[:, j, :],
                                                gb_fc2[:, j:j + 1])
                for th in range(2):
                    ts_ = slice(th * 512, (th + 1) * 512)
                    gact = gactp.tile([P, DM // P, 512], BF16, name="gact")
                    for q8 in range(8):  # wfc1 slab: 512 hidden features
                        wf1 = wf1q.pop(0) if wf1q else load_wf1(q8)
                        for fi in range(4):
                            fc = q8 * 4 + fi
                            ps = psM.tile([P, 512], F32, name="ps")
                            for d in range(NCH):
                                nc.tensor.matmul(ps,
                                                 wf1[:, d, fi * P:(fi + 1) * P],
                                                 h2[:, d, ts_],
                                                 start=(d == 0),
                                                 stop=(d == NCH - 1))
                            nc.scalar.activation(gact[:, fc, :], ps,
                                                 AF.Gelu_apprx_tanh,
                                                 bias=bfc1_c[:, fc:fc + 1])
                    for q2 in range(4):  # wfc2 slab: 256 out features
                        wf2 = wf2p.tile([P, DM // P, 256], BF16, name="wf2")
                        nc.sync.dma_start(out=wf2, in_=bass.AP(
                            tensor=wfc2, offset=q2 * 256,
                            ap=[[D, P], [P * D, DM // P], [1, 256]]))
                        for fi in range(2):
                            fc = q2 * 2 + fi
                            ps2 = psM2.tile([P, 512], F32, name="ps2")
                            for d32 in range(DM // P):
                                nc.tensor.matmul(ps2,
                                                 wf2[:, d32, fi * P:(fi + 1) * P],
                                                 gact[:, d32, :],
                                                 start=(d32 == 0),
                                                 stop=(d32 == DM // P - 1))
                            ot = evm.tile([P, 512], F32, name="otout")
                            nc.vector.scalar_tensor_tensor(
                                ot, ps2, adaT[:, 40 + fc:41 + fc],
                                x_res[:, fc, ts_], OP.mult, OP.add)
                            nc.sync.dma_start(out=out[fc * P:(fc + 1) * P, ts_],
                                              in_=ot)
            mlpw.close()
    nc.compile()
    return nc


_CACHE = {}


def _runner(nc, n_cores=8):
    import jax
    import numpy as _np
    from jax.sharding import Mesh, PartitionSpec, NamedSharding
    from jax.experimental.shard_map import shard_map
    from concourse.bass2jax import _bass_exec_p, install_neuronx_cc_hook, partition_id_tensor

    install_neuronx_cc_hook()
    in_names, out_names, out_avals = [], [], []
    partition_name = nc.partition_id_tensor.name if nc.partition_id_tensor else None
    for alloc in nc.m.functions[0].allocations:
        if not isinstance(alloc, mybir.MemoryLocationSet):
            continue
        nm = alloc.memorylocations[0].name
        if alloc.kind == "ExternalInput":
            if nm != partition_name:
                in_names.append(nm)
        elif alloc.kind == "ExternalOutput":
            out_names.append(nm)
            out_avals.append(jax.core.ShapedArray(tuple(alloc.tensor_shape),
                                                  mybir.dt.np(alloc.dtype)))

    def _body(*args):
        operands = list(args)
        if partition_name is not None:
            operands.append(partition_id_tensor())
        outs = _bass_exec_p.bind(
            *operands,
            out_avals=tuple(out_avals),
            in_names=tuple(in_names + [partition_name] if partition_name else in_names),
            out_names=tuple(out_names),
            lowering_input_output_aliases=(),
            sim_require_finite=False,
            sim_require_nnan=False,
            nc=nc,
        )
        return tuple(outs)

    devices = jax.devices()[:n_cores]
    mesh = Mesh(_np.asarray(devices), ("core",))
    fn = jax.jit(shard_map(_body, mesh=mesh,
                           in_specs=(PartitionSpec("core"),) * len(in_names),
                           out_specs=(PartitionSpec("core"),) * len(out_names),
                           check_rep=False))

    def run(in_maps):
        concat = [_np.concatenate([_np.asarray(m[n]) for m in in_maps], axis=0)
                  for n in in_names]
        args = [jax.device_put(c, NamedSharding(mesh, PartitionSpec("core")))
                for c in concat]
        outs = fn(*args)
        jax.block_until_ready(outs)
        res = []
        for c in range(n_cores):
            d = {}
            for i, nm in enumerate(out_names):
                full = _np.asarray(outs[i])
                d[nm] = full.reshape(n_cores, *out_avals[i].shape)[c]
            res.append(d)
        return res

    return run


def kernel(**inputs):
    """Full (unsharded) inputs -> full (B, T, D) float32 output."""
    if "nc" not in _CACHE:
        _CACHE["nc"] = build_dit(n_cores=8)
        _CACHE["run"] = _runner(_CACHE["nc"], 8)
    in_maps = host_prep(**inputs)
    results = _CACHE["run"](in_maps)
    return host_post(results)



# revision 71
# speedup vs baseline: 1.5081x; 1.5081x over previous
"""nn_DiTBlock on 8 TRN2 NeuronCores: data-parallel over batch (B=8), one
batch element per core. Self-contained: builds the Bass/Tile kernel, shards
inputs on the host (transpose/pack/cast only), runs SPMD via bass2jax/PJRT,
gathers and un-transposes the output.

v3 design (vs v2): all large GEMMs run in fp8e4m3 with DoubleRow perf mode
(weights host-scaled by 64 into fp8 range; the 64x passes through rmsnorm
invariantly for q/k, and is divided back out at each psum read elsewhere).
DoubleRow contracts two 128-row k-chunks per matmul, so the [P, nch, N] tile
layouts feed it directly. k's rmsnorm factor is folded into k_t itself
(instead of riding the softmax Exp scale), which lets each Exp cover a
2-bank psum score tile in one instruction with a constant -ln(16) bias that
keeps e within fp8 range (the 1/16 cancels in the softmax quotient).
Attention output is normalized once per head over a 2-bank psum tile. The
residual switches to bf16 after the attention branch."""

import numpy as np
from contextlib import ExitStack

import concourse.bass as bass
import concourse.mybir as mybir
import concourse.tile as tile
from concourse import bacc

F32 = mybir.dt.float32
F32R = mybir.dt.float32r
BF16 = mybir.dt.bfloat16
FP8 = mybir.dt.float8e4
AF = mybir.ActivationFunctionType
OP = mybir.AluOpType
DR = mybir.MatmulPerfMode.DoubleRow

B, T, D, H = 8, 1024, 1024, 16
HD = D // H          # 64
DM = 4 * D           # 4096
NCH = D // 128       # 8
P = 128
WS = 64.0            # host-side weight scale into fp8 range
IWS = 1.0 / WS
LN16 = 2.772588722239781  # softmax exp bias; cancels in the quotient


def host_prep(x, c, g1, g2, gq, gk, Wqkv, bqkv, Wproj, bproj,
              Wfc1, bfc1, Wfc2, bfc2, Wada, bada):
    f8 = mybir.dt.np(FP8)

    def packT8(W):  # (F, K) -> (K//128, 128, F) contiguous, fp8, x64
        Wt = np.ascontiguousarray(np.asarray(W, np.float32).T * WS).astype(f8)
        K, F = Wt.shape
        return np.ascontiguousarray(Wt.reshape(K // 128, 128, F))

    def packT8res(W):
        # fp8 of 32x the quantization residual of packT8(W)
        Wt = np.ascontiguousarray(np.asarray(W, np.float32).T * WS)
        R = (Wt - Wt.astype(f8).astype(np.float32)) * 32.0
        K, F = R.shape
        return np.ascontiguousarray(R.astype(f8).reshape(K // 128, 128, F))

    import ml_dtypes
    f32 = np.float32
    com = {
        "wqkv": packT8(Wqkv), "wproj": packT8(Wproj),
        "wfc1": packT8(Wfc1), "wfc2": packT8(Wfc2), "wada": packT8(Wada),
        "wadar": packT8res(Wada),
        "bqkv": np.asarray(bqkv, f32), "bproj": np.asarray(bproj, f32),
        "bfc1": np.asarray(bfc1, f32), "bfc2": np.asarray(bfc2, f32),
        "bada": np.asarray(bada, f32),
        "g": np.stack([np.asarray(g1)[0], np.asarray(g2)[0],
                       np.asarray(gq)[0], np.asarray(gk)[0]]).astype(f32),
    }
    in_maps = []
    for b in range(B):
        m = dict(com)
        m["xt"] = np.ascontiguousarray(
            np.asarray(x[b], f32).T.astype(ml_dtypes.bfloat16))
        m["cvec"] = np.asarray(c[b], f32)
        in_maps.append(m)
    return in_maps


def host_post(results):
    return np.ascontiguousarray(
        np.stack([r["out"].T for r in results]).astype(np.float32))


def col_ap(handle, nch):
    """DRAM (nch*128,) viewed as [128, nch]: tile[p, ch] = v[ch*128+p]."""
    return bass.AP(tensor=handle, offset=0, ap=[[1, P], [P, nch]])


def bc_ap(handle, n, offset=0):
    """DRAM (n,) broadcast-read to [128, n] (partition stride 0)."""
    return bass.AP(tensor=handle, offset=offset, ap=[[0, P], [1, n]])


def wslab_ap(handle, F, c0, ncols, nch=NCH, doff=0):
    """DRAM weight (nch_tot, 128, F) -> SBUF [128, nch, ncols] in one DMA:
    tile[p, d, c] = w[doff+d, p, c0+c]."""
    return bass.AP(tensor=handle, offset=doff * P * F + c0,
                   ap=[[F, P], [P * F, nch], [1, ncols]])


def _pin_exp_ln_table():
    """Make every Exp/Ln activation resolve to the one act-func table that
    holds both ('natural_log_exp_and_others'), so the rsqrt-via-exp(ln) and
    the attention Exps never force a LoadActFuncSet between head pairs."""
    import functools
    import concourse.hw_specs as hws
    import concourse.bacc as bacc_mod
    if getattr(hws, "_exp_ln_pinned", False):
        return
    orig = hws.get_activation_tables

    @functools.cache
    def patched(module_arch):
        tabs = {k: set(v) for k, v in orig(module_arch).items()}
        keep = "natural_log_exp_and_others"
        if keep in tabs:
            for name, s in tabs.items():
                if name != keep:
                    s.discard(AF.Exp)
                    s.discard(AF.Ln)
        return tabs

    hws.get_activation_tables = patched
    bacc_mod.get_activation_tables = patched
    hws._exp_ln_pinned = True


def build_dit(n_cores=8, repeat=1):
    _pin_exp_ln_table()
    nc = bacc.Bacc("TRN2", target_bir_lowering=False, debug=False,
                   num_devices=n_cores)

    xt = nc.dram_tensor("xt", [D, T], BF16, kind="ExternalInput")
    cin = nc.dram_tensor("cvec", [D], F32, kind="ExternalInput")
    g = nc.dram_tensor("g", [4], F32, kind="ExternalInput")
    wqkv = nc.dram_tensor("wqkv", [NCH, P, 3 * D], FP8, kind="ExternalInput")
    wproj = nc.dram_tensor("wproj", [NCH, P, D], FP8, kind="ExternalInput")
    wfc1 = nc.dram_tensor("wfc1", [NCH, P, DM], FP8, kind="ExternalInput")
    wfc2 = nc.dram_tensor("wfc2", [DM // P, P, D], FP8, kind="ExternalInput")
    wada = nc.dram_tensor("wada", [NCH, P, 6 * D], FP8, kind="ExternalInput")
    wadar = nc.dram_tensor("wadar", [NCH, P, 6 * D], FP8, kind="ExternalInput")
    bqkv = nc.dram_tensor("bqkv", [3 * D], F32, kind="ExternalInput")
    bproj = nc.dram_tensor("bproj", [D], F32, kind="ExternalInput")
    bfc1 = nc.dram_tensor("bfc1", [DM], F32, kind="ExternalInput")
    bfc2 = nc.dram_tensor("bfc2", [D], F32, kind="ExternalInput")
    bada = nc.dram_tensor("bada", [6 * D], F32, kind="ExternalInput")
    out = nc.dram_tensor("out", [D, T], F32, kind="ExternalOutput")

    with tile.TileContext(nc, pool_alloc_mode="queue") as tc:
      for _rep in range(repeat):
        with ExitStack() as X:
            const = X.enter_context(tc.tile_pool(name="const", bufs=1))
            dram = X.enter_context(tc.tile_pool(name="dram", bufs=1, space="DRAM"))

            # ---------------- input DMAs (SP queue order = priority) -------
            c_pm = const.tile([P, NCH], F32)
            nc.sync.dma_start(out=c_pm, in_=col_ap(cin, NCH))
            g_bc = const.tile([P, 4], F32)
            nc.sync.dma_start(out=g_bc, in_=bc_ap(g, 4))
            bqkv_c = const.tile([P, 3 * D // P], F32)
            nc.sync.dma_start(out=bqkv_c, in_=col_ap(bqkv, 3 * D // P))

            badaT = const.tile([P, 48], F32)
            nc.sync.dma_start(out=badaT, in_=col_ap(bada, 48))
            bfc1_c = const.tile([P, DM // P], F32)
            nc.sync.dma_start(out=bfc1_c, in_=col_ap(bfc1, DM // P))
            bproj_c = const.tile([P, NCH], F32)
            nc.sync.dma_start(out=bproj_c, in_=col_ap(bproj, NCH))
            bfc2_c = const.tile([P, NCH], F32)
            nc.sync.dma_start(out=bfc2_c, in_=col_ap(bfc2, NCH))
            vbias_bc = const.tile([P, D], BF16)
            nc.gpsimd.dma_start(out=vbias_bc, in_=bc_ap(bqkv, D, offset=2 * D))

            # x first: it gates norm1 -> mod1 -> everything
            residf = X.enter_context(tc.tile_pool(name="residf", bufs=1))
            x_res = residf.tile([P, NCH, T], BF16)
            for j2 in range(4):  # chunked so norm1 squares can pipeline
                nc.sync.dma_start(
                    out=x_res[:, 2 * j2:2 * j2 + 2, :],
                    in_=bass.AP(tensor=xt, offset=2 * j2 * P * T,
                                ap=[[T, P], [P * T, 2], [1, T]]))

            arp = X.enter_context(tc.tile_pool(name="arp", bufs=2))
            adas = ExitStack()
            wadap = adas.enter_context(tc.tile_pool(name="wadap", bufs=1))
            wada_sb = wadap.tile([P, NCH, 2048], FP8)
            wadar_sb = wadap.tile([P, NCH, 2048], FP8)
            nc.sync.dma_start(out=wada_sb, in_=wslab_ap(wada, 6 * D, 0, 2048))
            nc.sync.dma_start(out=wadar_sb,
                              in_=wslab_ap(wadar, 6 * D, 0, 2048))

            wqkp = ExitStack()
            wqk = wqkp.enter_context(
                tc.tile_pool(name="wqkp", bufs=1, side="right")) \
                .tile([P, NCH, 3 * D], FP8)
            nc.sync.dma_start(out=wqk[:, :, 0:2048],
                              in_=wslab_ap(wqkv, 3 * D, 0, 2048))
            nc.sync.dma_start(out=wqk[:, :, 2048:3072],
                              in_=wslab_ap(wqkv, 3 * D, 2048, 1024))


            # ---------------- constants ----------------
            gsq = const.tile([P, 4], F32)
            nc.vector.tensor_tensor(gsq, g_bc, g_bc, OP.mult)
            ginv2 = const.tile([P, 4], F32)
            nc.vector.reciprocal(ginv2, gsq)
            scl_n1 = const.tile([P, 1], F32)
            nc.vector.tensor_scalar_mul(scl_n1, ginv2[:, 0:1], 1.0 / D)
            scl_n2 = const.tile([P, 1], F32)
            nc.vector.tensor_scalar_mul(scl_n2, ginv2[:, 1:2], 1.0 / D)
            bqkv64 = const.tile([P, 16], F32)
            nc.vector.tensor_scalar_mul(bqkv64, bqkv_c[:, 0:16], WS)

            # DR stationary pair-stride must be 16B-aligned -> pad to 16
            ones2_t = const.tile([P, 2, 16], FP8)
            nc.gpsimd.memset(ones2_t, 1.0)
            ones2 = ones2_t[:, :, 0:1]
            # per-half ones carrying the q/k rmsnorm scale factors
            onesh_q = const.tile([P, 2], BF16)
            nc.gpsimd.memset(onesh_q, 0.0)
            nc.gpsimd.memset(onesh_q[0:64, 0:1], 1.0)
            nc.gpsimd.memset(onesh_q[64:128, 1:2], 1.0)
            onesh_k = const.tile([P, 2], BF16)
            nc.vector.tensor_scalar_mul(onesh_k, onesh_q,
                                        ginv2[:, 3:4])
            nc.vector.tensor_scalar_mul(onesh_k, onesh_k, 1.0 / HD)
            nc.vector.tensor_scalar_mul(onesh_q, onesh_q, ginv2[:, 2:3])

            nln16 = const.tile([P, 1], F32)
            nc.gpsimd.memset(nln16, -LN16)

            # silu(c) in fp8 + 32x-scaled fp8 residual (two-lane precision)
            silu_f = const.tile([P, NCH], F32)
            nc.scalar.activation(silu_f, c_pm, AF.Silu)
            cs_pm_t = const.tile([P, NCH, 16], FP8)
            nc.scalar.copy(
                cs_pm_t[:, :, 0:1].rearrange("p c o -> p (c o)"), silu_f)
            cs_pm = cs_pm_t[:, :, 0:1]
            rs_f = const.tile([P, NCH], F32)
            nc.vector.scalar_tensor_tensor(
                rs_f, cs_pm.rearrange("p c o -> p (c o)"), -1.0, silu_f,
                OP.mult, OP.add)
            rs32_t = const.tile([P, NCH, 16], FP8)
            nc.vector.tensor_scalar_mul(
                rs32_t[:, :, 0:1].rearrange("p c o -> p (c o)"), rs_f, 32.0)
            rs32 = rs32_t[:, :, 0:1]

            # ---------------- adaLN ----------------
            # cols: shift_msa 0:8 | scale_msa 8:16 | gate_msa 16:24
            #       shift_mlp 24:32 | scale_mlp 32:40 | gate_mlp 40:48
            ada_scr = dram.tile([1, 6 * D], F32)
            adaT = const.tile([P, 48], F32)

            def ada_block(pa, par, wa, war, cs, ns):
                """one 512-col adaLN chunk: main + 32x-scaled corrections
                (rsil*W8 + silu8*Wres), staged and written to ada_scr."""
                for d2 in range(4):
                    nc.tensor.matmul(pa, cs_pm[:, 2 * d2:2 * d2 + 2, :],
                                     wa[:, 2 * d2:2 * d2 + 2, cs],
                                     start=(d2 == 0), stop=(d2 == 3),
                                     perf_mode=DR)
                for d2 in range(4):
                    nc.tensor.matmul(par, rs32[:, 2 * d2:2 * d2 + 2, :],
                                     wa[:, 2 * d2:2 * d2 + 2, cs],
                                     start=(d2 == 0), stop=False,
                                     perf_mode=DR)
                for d2 in range(4):
                    nc.tensor.matmul(par, cs_pm[:, 2 * d2:2 * d2 + 2, :],
                                     war[:, 2 * d2:2 * d2 + 2, cs],
                                     start=False, stop=(d2 == 3),
                                     perf_mode=DR)
                stage = arp.tile([1, 512], F32, name="stage")
                nc.scalar.activation(stage, pa, AF.Identity, scale=IWS)
                nc.vector.scalar_tensor_tensor(
                    stage, par, IWS / 32.0, stage, OP.mult, OP.add)
                nc.sync.dma_start(out=ada_scr[:, ns], in_=stage)

            with tc.tile_pool(name="psA", bufs=2, space="PSUM") as psA:
                for nb in range(4):  # msa half: cols 0:2048
                    pa = psA.tile([1, 512], F32, name="pa", tag="pa")
                    par = psA.tile([1, 512], F32, name="par", tag="par")
                    cs = slice(nb * 512, (nb + 1) * 512)
                    ada_block(pa, par, wada_sb, wadar_sb, cs, cs)
            # shift/scale_msa (cols 0:16) read back early to unblock mod1;
            # the mlp half is computed later between attention exps.
            nc.sync.dma_start(out=adaT[:, 0:16], in_=bass.AP(
                tensor=ada_scr.tensor, offset=0, ap=[[1, P], [P, 16]]))
            nc.vector.tensor_tensor(adaT[:, 0:16], adaT[:, 0:16],
                                    badaT[:, 0:16], OP.add)
            nc.vector.tensor_scalar_add(adaT[:, 8:16], adaT[:, 8:16], 1.0)
            gb_proj = const.tile([P, NCH], F32)
            gb_fc2 = const.tile([P, NCH], F32)
            gpj64 = const.tile([P, NCH], F32)
            gml64 = const.tile([P, NCH], F32)

            def ada_tail():
                """read back mlp-half adaLN cols and derive gate tiles."""
                nc.sync.dma_start(out=adaT[:, 16:48], in_=bass.AP(
                    tensor=ada_scr.tensor, offset=2048, ap=[[1, P], [P, 32]]))
                nc.vector.tensor_tensor(adaT[:, 16:48], adaT[:, 16:48],
                                        badaT[:, 16:48], OP.add)
                nc.vector.tensor_scalar_add(adaT[:, 32:40], adaT[:, 32:40],
                                            1.0)
                nc.vector.tensor_tensor(gb_proj, adaT[:, 16:24], bproj_c,
                                        OP.mult)
                nc.vector.tensor_tensor(gb_fc2, adaT[:, 40:48], bfc2_c,
                                        OP.mult)
                # gates pre-divided by the weight scale for psum reads
                nc.vector.tensor_scalar_mul(gpj64, adaT[:, 16:24], IWS)
                nc.vector.tensor_scalar_mul(gml64, adaT[:, 40:48], IWS)

            def norm_sums(src, scl, nrm, sqp, psN):
                """rmsnorm 1/||.|| factor of src, broadcast to [P, T]."""
                pss = [psN.tile([1, 512], F32, name="pss") for _ in range(2)]
                for j2 in range(4):
                    xsq = sqp.tile([P, 2, T], FP8, name="xsq")
                    nc.scalar.activation(
                        xsq.rearrange("p c t -> p (c t)"),
                        src[:, 2 * j2:2 * j2 + 2, :].rearrange(
                            "p c t -> p (c t)"), AF.Square)
                    for t2 in range(2):
                        nc.tensor.matmul(pss[t2], ones2,
                                         xsq[:, :, t2 * 512:(t2 + 1) * 512],
                                         start=(j2 == 0), stop=(j2 == 3),
                                         perf_mode=DR)
                rr = nrm.tile([1, T], F32, name="rr", tag="rr")
                rinv = nrm.tile([1, T], BF16, name="rinv", tag="ri")
                for t2 in range(2):
                    ts2 = slice(t2 * 512, (t2 + 1) * 512)
                    nc.scalar.activation(rr[:, ts2], pss[t2], AF.Ln,
                                         scale=scl[0:1, :])
                    nc.scalar.activation(rinv[:, ts2], rr[:, ts2], AF.Exp,
                                         scale=-0.5)
                rbc = nrm.tile([P, T], BF16, name="rbc", tag="rb")
                for t2 in range(2):
                    ts2 = slice(t2 * 512, (t2 + 1) * 512)
                    nc.gpsimd.partition_broadcast(rbc[:, ts2], rinv[:, ts2])
                return rbc

            def mod_apply(src, rbc, sh_col, sc_col, h_out, ts, xnp,
                          on_act=False):
                # optionally scale+shift on Act (when it would idle anyway)
                n = ts.stop - ts.start
                for j in range(NCH):
                    xn = xnp.tile([P, n], BF16, name="xn")
                    nc.vector.tensor_tensor(xn, src[:, j, ts], rbc[:, ts],
                                            OP.mult)
                    if on_act:
                        nc.scalar.activation(h_out[:, j, ts], xn, AF.Identity,
                                             bias=adaT[:, sh_col + j:
                                                       sh_col + j + 1],
                                             scale=adaT[:, sc_col + j:
                                                        sc_col + j + 1])
                    else:
                        nc.vector.tensor_scalar(
                            h_out[:, j, ts], xn,
                            adaT[:, sc_col + j:sc_col + j + 1],
                            adaT[:, sh_col + j:sh_col + j + 1],
                            OP.mult, OP.add)

            def norm_modulate(src, scl, sh_col, sc_col, h_out):
                with tc.tile_pool(name="sqp", bufs=3) as sqp, \
                     tc.tile_pool(name="psN", bufs=2, space="PSUM") as psN, \
                     tc.tile_pool(name="nrm", bufs=1) as nrm, \
                     tc.tile_pool(name="xnp", bufs=3) as xnp:
                    rbc = norm_sums(src, scl, nrm, sqp, psN)
                    for th in range(2):  # halves so qkv can start earlier
                        mod_apply(src, rbc, sh_col, sc_col, h_out,
                                  slice(th * 512, (th + 1) * 512), xnp,
                                  on_act=True)

            att = ExitStack()

            h1p = att.enter_context(tc.tile_pool(name="h1p", bufs=1, side="right"))
            h1 = h1p.tile([P, NCH, T], FP8)
            vxp = att.enter_context(tc.tile_pool(name="vxp", bufs=1, side="right"))
            vx = vxp.tile([P, NCH, H, HD + 1], FP8)  # [kt_chunk][head][(v,1)]
            nc.gpsimd.memset(vx[:, :, :, HD:HD + 1], 1.0)

            # mlp-half ada weights: consumed by attention fillers, then freed
            adas2 = ExitStack()
            wadap2 = adas2.enter_context(
                tc.tile_pool(name="wadap2", bufs=1, side="right"))
            wada_m = wadap2.tile([P, NCH, 4096], FP8)
            wadar_m = wadap2.tile([P, NCH, 4096], FP8)
            for q2 in range(2):
                nc.sync.dma_start(
                    out=wada_m[:, :, q2 * 2048:(q2 + 1) * 2048],
                    in_=wslab_ap(wada, 6 * D, 2048 + q2 * 2048, 2048))
                nc.sync.dma_start(
                    out=wadar_m[:, :, q2 * 2048:(q2 + 1) * 2048],
                    in_=wslab_ap(wadar, 6 * D, 2048 + q2 * 2048, 2048))

            # ------------ norm1 + modulate ------------
            norm_modulate(x_res, scl_n1, 0, 8, h1)
            adas.close()

            # ------------ interleaved qk-gen + attention per head pair ----
            wpjp = ExitStack()
            wpj = wpjp.enter_context(
                tc.tile_pool(name="wpjp", bufs=1)) \
                .tile([P, NCH, D], FP8)
            oT_sb = wpjp.enter_context(tc.tile_pool(name="otp", bufs=1)) \
                .tile([P, NCH, T], FP8)

            qp_ = att.enter_context(tc.tile_pool(name="qp_", bufs=1))
            kp_ = att.enter_context(tc.tile_pool(name="kp_", bufs=1))
            q_t = qp_.tile([P, NCH, T], BF16)
            k_t = kp_.tile([P, NCH, T], BF16)

            with tc.tile_pool(name="sqq", bufs=2) as sqq, \
                 tc.tile_pool(name="psDR", bufs=1, space="PSUM") as psDR, \
                 tc.tile_pool(name="nrq", bufs=2) as nrq, \
                 tc.tile_pool(name="pscp", bufs=2, space="PSUM") as pscp, \
                 tc.tile_pool(name="psO", bufs=1, space="PSUM") as psO, \
                 tc.tile_pool(name="esp", bufs=5) as esp, \
                 tc.tile_pool(name="onp", bufs=2) as onp:

                nc.sync.dma_start(out=wpj, in_=wslab_ap(wproj, D, 0, D))

                def vgen1(nh, t8):
                    """v for heads nh*8..nh*8+7, key-token chunk t8 (pv
                    tiles share the psd bank with qkgen)."""
                    pv = psDR.tile([P, 512], F32, name="pv", tag="psd")
                    for d2 in range(4):
                        nc.tensor.matmul(
                            pv,
                            h1[:, 2 * d2:2 * d2 + 2, t8 * P:(t8 + 1) * P],
                            wqk[:, 2 * d2:2 * d2 + 2,
                                2 * D + nh * 512:2 * D + (nh + 1) * 512],
                            start=(d2 == 0), stop=(d2 == 3),
                            perf_mode=DR)
                    nc.vector.scalar_tensor_tensor(
                        vx[:, t8, nh * 8:(nh + 1) * 8, 0:HD],
                        pv.rearrange("p (h e) -> p h e", e=HD),
                        IWS,
                        vbias_bc[:, nh * 512:(nh + 1) * 512].rearrange(
                            "p (h e) -> p h e", e=HD),
                        OP.mult, OP.add)

                def qk_mm(j, gi, nt, pcol, sq):
                    """One quarter of head-pair j's qkv generation."""
                    fc = j if gi == 0 else 8 + j
                    tgt = q_t if gi == 0 else k_t
                    onesh = onesh_q if gi == 0 else onesh_k
                    ts2 = slice(nt * 512, (nt + 1) * 512)
                    psd = psDR.tile([P, 512], F32, name="psd", tag="psd")
                    for d2 in range(4):
                        nc.tensor.matmul(
                            psd,
                            wqk[:, 2 * d2:2 * d2 + 2, fc * P:(fc + 1) * P],
                            h1[:, 2 * d2:2 * d2 + 2, ts2],
                            start=(d2 == 0), stop=(d2 == 3),
                            perf_mode=DR)
                    nc.vector.tensor_scalar_add(tgt[:, j, ts2], psd,
                                                bqkv64[:, fc:fc + 1])
                    nc.vector.tensor_tensor(sq[:, ts2], tgt[:, j, ts2],
                                            tgt[:, j, ts2], OP.mult)
                    for tc2 in range(4):
                        tci = nt * 4 + tc2
                        cb = tci * 4 + gi * 2
                        nc.tensor.matmul(pcol[:, cb:cb + 2],
                                         sq[:, tci * P:(tci + 1) * P],
                                         onesh, start=True, stop=True)

                def qk_rsqrt(j, pcol):
                    """rsqrt via exp(-0.5*ln(x)) for q and k in one pass,
                    then row-ize each (q/k, hf) group into natural token
                    order (one strided DMA per group)."""
                    rr = nrq.tile([P, 32], F32, name="rr", tag="rr")
                    nc.scalar.activation(rr, pcol, AF.Ln)
                    rinv_c = nrq.tile([P, 32], BF16, name="rinv_c", tag="ri")
                    nc.scalar.activation(rinv_c, rr, AF.Exp, scale=-0.5)
                    rq4n = nrq.tile([1, P, 32], BF16, name="rq4n", tag="rq",
                                    bufs=1)
                    nc.sync.dma_start(
                        out=bass.AP(tensor=rq4n.tensor, offset=rq4n.offset,
                                    ap=[[1, 1], [32, P], [1, 32]]),
                        in_=rinv_c)
                    return rq4n.rearrange("o p (t g) -> o p t g", g=4)

                def qk_norm(j, gi, hf, rq4v):
                    """in-place normalize one 64-row half of q_t/k_t
                    (p-major permuted row; the TT un-permutes via
                    rearranged APs)."""
                    tgt = q_t if gi == 0 else k_t
                    rbc = nrq.tile([P, T], BF16, name="rbc", tag="rb")
                    nc.gpsimd.partition_broadcast(rbc,
                                                  rq4v[:, :, :, 2 * gi + hf])
                    hs = slice(64 * hf, 64 * (hf + 1))
                    nc.vector.tensor_tensor(
                        tgt[hs, j, :].rearrange("p (tc pp) -> p tc pp",
                                                tc=NCH),
                        tgt[hs, j, :].rearrange("p (tc pp) -> p tc pp",
                                                tc=NCH),
                        rbc.rearrange("p (pp tc) -> p tc pp", tc=NCH)[hs, :, :],
                        OP.mult)

                def qkgen_pieces(j):
                    # col layout: tc-major, [q hf0, q hf1, k hf0, k hf1] / tc
                    pcol = psDR.tile([P, 32], F32, name="pcol", tag="pcol",
                                     bufs=1)
                    sqs = [sqq.tile([P, T], BF16, name="sq") for _ in range(2)]
                    box = {}

                    def mk_mm(gi, nt):
                        return lambda: qk_mm(j, gi, nt, pcol, sqs[gi])

                    def mk_rsqrt():
                        def f():
                            box["rq4n"] = qk_rsqrt(j, pcol)
                        return f

                    def mk_norm(gi, hf):
                        return lambda: qk_norm(j, gi, hf, box["rq4n"])

                    return [mk_mm(0, 0), mk_mm(0, 1), mk_mm(1, 0),
                            mk_mm(1, 1), mk_rsqrt(),
                            mk_norm(0, 0), mk_norm(0, 1),
                            mk_norm(1, 0), mk_norm(1, 1)]

                def att_head(j, hf, fillers):
                    h = 2 * j + hf
                    rq = slice(64 * hf, 64 * (hf + 1))
                    po = psO.tile([65, 2, 512], F32, name="po")
                    for qt in range(2):
                        qs = slice(qt * 512, (qt + 1) * 512)
                        for t4 in range(4):
                            psc = pscp.tile([P, 2, 512], F32, name="psc")
                            for t2 in range(2):
                                ktc = 2 * t4 + t2
                                nc.tensor.matmul(
                                    psc[:, t2, :],
                                    k_t[rq, j, ktc * P:(ktc + 1) * P],
                                    q_t[rq, j, qs],
                                    start=True, stop=True)
                            e = esp.tile([P, 2, 512], FP8, name="es")
                            nc.scalar.activation(
                                e.rearrange("p c t -> p (c t)"),
                                psc.rearrange("p c t -> p (c t)"),
                                AF.Exp, bias=nln16[:, 0:1])
                            nc.tensor.matmul(po[:, qt, :],
                                             vx[:, 2 * t4:2 * t4 + 2, h, :],
                                             e,
                                             start=(t4 == 0), stop=(t4 == 3),
                                             perf_mode=DR)
                            if fillers:
                                fillers.popleft()()
                    rs = onp.tile([65, 2, 512], BF16, name="rs", tag="rs")
                    with nc.allow_low_precision(reason="softmax denom"):
                        nc.vector.reciprocal(
                            rs[64:65, :, :].rearrange("p c t -> p (c t)"),
                            po[64:65, :, :].rearrange("p c t -> p (c t)"))
                    rs0 = onp.tile([1, T], BF16, name="rs0", tag="rs0",
                                   bufs=1)
                    nc.sync.dma_start(
                        out=rs0,
                        in_=rs[64:65, :, :].rearrange("p c t -> p (c t)"))
                    rsb = onp.tile([64, T], BF16, name="rsb", tag="rb",
                                   bufs=1)
                    nc.gpsimd.partition_broadcast(rsb, rs0)
                    on = onp.tile([64, 2, 512], FP8, name="on", tag="on")
                    nc.vector.tensor_tensor(
                        on, po[0:64, :, :],
                        rsb.rearrange("p (c t) -> p c t", c=2), OP.mult)
                    nc.sync.dma_start(
                        out=oT_sb[rq, j, :],
                        in_=on.rearrange("p c t -> p (c t)"))

                from collections import deque
                wf1q = []

                def mk_v(nh, t8):
                    return lambda: vgen1(nh, t8)

                for f in qkgen_pieces(0):
                    f()
                for t8 in range(4):
                    vgen1(0, t8)

                def mk_v2(t8):
                    def f():
                        vgen1(0, t8)
                        vgen1(0, t8 + 1)
                    return f
                fillers = deque((mk_v2(4), mk_v2(6)))
                fillers.extend(qkgen_pieces(1))
                def mk_ada(nb):
                    def f():
                        pa = psDR.tile([1, 512], F32, name="pa", tag="psd")
                        par = psDR.tile([1, 512], F32, name="par", tag="pcol",
                                        bufs=1)
                        ada_block(pa, par, wada_m, wadar_m,
                                  slice(nb * 512, (nb + 1) * 512),
                                  slice(2048 + nb * 512, 2048 + (nb + 1) * 512))
                    return f

                for j in range(NCH):
                    if j == 1:
                        fillers.extend(mk_ada(nb) for nb in range(4))
                    elif j == 2:
                        fillers.extend(mk_ada(nb) for nb in range(4, 8))
                    elif j == 3:
                        fillers.append(ada_tail)
                        # vx for heads 8..15, spread between exps
                        fillers.extend(mk_v(1, t8) for t8 in range(NCH))
                    elif j == 4:
                        adas2.close()
                    elif j == 5:
                        # gate_msa * bproj pre-added to residual (Pool)
                        def mk_gb(jj):
                            return lambda: nc.gpsimd.tensor_scalar_add(
                                x_res[:, jj, :], x_res[:, jj, :],
                                gb_proj[:, jj:jj + 1])
                        fillers.extend(mk_gb(jj) for jj in range(NCH))
                    att_head(j, 0, fillers)
                    att_head(j, 1, fillers)
                    while fillers:
                        fillers.popleft()()
                    if j + 2 < NCH:
                        fillers = deque(qkgen_pieces(j + 2))

            att.close()  # free h1, vx, q_t, k_t (oT_sb, x_res stay)
            wqkp.close()

            # MLP weights stream in while proj/norm2 run (chunked so any
            # straggling attention-tail DMA isn't stuck behind them)
            mlpw = ExitStack()
            wf1p = mlpw.enter_context(tc.tile_pool(name="wf1p", bufs=1))
            wf1 = wf1p.tile([P, NCH, DM], FP8, name="wf1")
            wf2p = mlpw.enter_context(tc.tile_pool(name="wf2p", bufs=1))
            wf2 = wf2p.tile([P, DM // P, D], FP8, name="wf2")
            for q4 in range(4):
                nc.sync.dma_start(
                    out=wf1[:, :, q4 * 1024:(q4 + 1) * 1024],
                    in_=wslab_ap(wfc1, DM, q4 * 1024, 1024))
            for q4 in range(4):
                nc.sync.dma_start(
                    out=wf2[:, 8 * q4:8 * (q4 + 1), :],
                    in_=bass.AP(tensor=wfc2, offset=8 * q4 * P * D,
                                ap=[[D, P], [P * D, NCH], [1, D]]))

            # ---- proj + residual, fused with norm2's square-sums; then
            # ---- modulate + MLP with fc2(th0) interleaved into fc1(th1).
            with tc.tile_pool(name="h2p", bufs=1) as h2p, \
                 tc.tile_pool(name="gactp", bufs=2) as gactp, \
                 tc.tile_pool(name="sqp", bufs=3) as sqp2, \
                 tc.tile_pool(name="psN", bufs=2, space="PSUM") as psN2, \
                 tc.tile_pool(name="nrm", bufs=1) as nrm2, \
                 tc.tile_pool(name="xnp", bufs=3) as xnp2, \
                 tc.tile_pool(name="evm", bufs=4) as evm:
                h2 = h2p.tile([P, NCH, T], FP8)
                pss2 = [psN2.tile([1, 512], F32, name="pss") for _ in range(2)]
                with tc.tile_pool(name="psP", bufs=3, space="PSUM") as psP:
                    for fc in range(NCH):
                        ps = psP.tile([P, 2, 512], F32, name="ps")
                        for nt in range(2):
                            for d2 in range(4):
                                nc.tensor.matmul(
                                    ps[:, nt, :],
                                    wpj[:, 2 * d2:2 * d2 + 2,
                                        fc * P:(fc + 1) * P],
                                    oT_sb[:, 2 * d2:2 * d2 + 2,
                                          nt * 512:(nt + 1) * 512],
                                    start=(d2 == 0), stop=(d2 == 3),
                                    perf_mode=DR)
                        nc.vector.scalar_tensor_tensor(
                            x_res[:, fc, :],
                            ps.rearrange("p c t -> p (c t)"),
                            gpj64[:, fc:fc + 1],
                            x_res[:, fc, :],
                            OP.mult, OP.add)
                        if fc % 2 == 1:  # norm2 square-sums ride along
                            j2 = fc // 2
                            xsq = sqp2.tile([P, 2, T], FP8, name="xsq")
                            nc.scalar.activation(
                                xsq.rearrange("p c t -> p (c t)"),
                                x_res[:, 2 * j2:2 * j2 + 2, :].rearrange(
                                    "p c t -> p (c t)"), AF.Square)
                            for t2 in range(2):
                                nc.tensor.matmul(
                                    pss2[t2], ones2,
                                    xsq[:, :, t2 * 512:(t2 + 1) * 512],
                                    start=(j2 == 0), stop=(j2 == 3),
                                    perf_mode=DR)
                rr2 = nrm2.tile([1, T], F32, name="rr", tag="rr")
                rinv2 = nrm2.tile([1, T], BF16, name="rinv", tag="ri")
                rbc2 = nrm2.tile([P, T], BF16, name="rbc", tag="rb")
                for t2 in range(2):
                    ts2 = slice(t2 * 512, (t2 + 1) * 512)
                    nc.scalar.activation(rr2[:, ts2], pss2[t2], AF.Ln,
                                         scale=scl_n2[0:1, :])
                    nc.scalar.activation(rinv2[:, ts2], rr2[:, ts2], AF.Exp,
                                         scale=-0.5)
                    nc.gpsimd.partition_broadcast(rbc2[:, ts2],
                                                  rinv2[:, ts2])
                for th in range(2):
                    mod_apply(x_res, rbc2, 24, 32, h2,
                              slice(th * 512, (th + 1) * 512), xnp2)
                # gate_mlp * bfc2 pre-added to residual (overlaps fc1)
                for j in range(NCH):
                    nc.gpsimd.tensor_scalar_add(x_res[:, j, :],
                                                x_res[:, j, :],
                                                gb_fc2[:, j:j + 1])

                with tc.tile_pool(name="psM", bufs=3, space="PSUM") as psM, \
                     tc.tile_pool(name="psM2", bufs=3, space="PSUM") as psM2:
                    gacts = [gactp.tile([P, DM // P, 512], FP8, name="gact")
                             for _ in range(2)]

                    def fc1_chunk(th, q8):
                        ts_ = slice(th * 512, (th + 1) * 512)
                        for fi in range(4):
                            fc = q8 * 4 + fi
                            ps = psM.tile([P, 512], F32, name="ps")
                            for d2 in range(4):
                                nc.tensor.matmul(
                                    ps,
                                    wf1[:, 2 * d2:2 * d2 + 2,
                                        fc * P:(fc + 1) * P],
                                    h2[:, 2 * d2:2 * d2 + 2, ts_],
                                    start=(d2 == 0), stop=(d2 == 3),
                                    perf_mode=DR)
                            nc.scalar.activation(gacts[th][:, fc, :], ps,
                                                 AF.Gelu_apprx_tanh,
                                                 bias=bfc1_c[:, fc:fc + 1],
                                                 scale=IWS)

                    def fc2_chunk(th, fc):
                        ts_ = slice(th * 512, (th + 1) * 512)
                        ps2 = psM2.tile([P, 512], F32, name="ps2")
                        for d2 in range(DM // P // 2):
                            nc.tensor.matmul(
                                ps2,
                                wf2[:, 2 * d2:2 * d2 + 2,
                                    fc * P:(fc + 1) * P],
                                gacts[th][:, 2 * d2:2 * d2 + 2, :],
                                start=(d2 == 0),
                                stop=(d2 == DM // P // 2 - 1),
                                perf_mode=DR)
                        ot = evm.tile([P, 512], F32, name="otout")
                        nc.vector.scalar_tensor_tensor(
                            ot, ps2, gml64[:, fc:fc + 1],
                            x_res[:, fc, ts_], OP.mult, OP.add)
                        nc.sync.dma_start(out=out[fc * P:(fc + 1) * P, ts_],
                                          in_=ot)

                    for q8 in range(8):
                        fc1_chunk(0, q8)
                    for i in range(8):
                        fc2_chunk(0, i)
                        fc1_chunk(1, i)
                    for i in range(8):
                        fc2_chunk(1, i)
            mlpw.close()
            wpjp.close()
    nc.compile()
    return nc


_CACHE = {}


def _runner(nc, n_cores=8):
    import jax
    import numpy as _np
    from jax.sharding import Mesh, PartitionSpec, NamedSharding
    from jax.experimental.shard_map import shard_map
    from concourse.bass2jax import _bass_exec_p, install_neuronx_cc_hook, partition_id_tensor

    install_neuronx_cc_hook()
    in_names, out_names, out_avals = [], [], []
    partition_name = nc.partition_id_tensor.name if nc.partition_id_tensor else None
    for alloc in nc.m.functions[0].allocations:
        if not isinstance(alloc, mybir.MemoryLocationSet):
            continue
        nm = alloc.memorylocations[0].name
        if alloc.kind == "ExternalInput":
            if nm != partition_name:
                in_names.append(nm)
        elif alloc.kind == "ExternalOutput":
            out_names.append(nm)
            out_avals.append(jax.core.ShapedArray(tuple(alloc.tensor_shape),
                                                  mybir.dt.np(alloc.dtype)))

    def _body(*args):
        operands = list(args)
        if partition_name is not None:
            operands.append(partition_id_tensor())
        outs = _bass_exec_p.bind(
            *operands,
            out_avals=tuple(out_avals),
            in_names=tuple(in_names + [partition_name] if partition_name else in_names),
            out_names=tuple(out_names),
            lowering_input_output_aliases=(),
            sim_require_finite=False,
            sim_require_nnan=False,
            nc=nc,
        )
        return tuple(outs)

    devices = jax.devices()[:n_cores]
    mesh = Mesh(_np.asarray(devices), ("core",))
    fn = jax.jit(shard_map(_body, mesh=mesh,
                           in_specs=(PartitionSpec("core"),) * len(in_names),
                           out_specs=(PartitionSpec("core"),) * len(out_names),
                           check_rep=False))

    def run(in_maps):
        concat = [_np.concatenate([_np.asarray(m[n]) for m in in_maps], axis=0)
                  for n in in_names]
        args = [jax.device_put(c, NamedSharding(mesh, PartitionSpec("core")))
                for c in concat]
        outs = fn(*args)
        jax.block_until_ready(outs)
        res = []
        for c in range(n_cores):
            d = {}
            for i, nm in enumerate(out_names):
                full = _np.asarray(outs[i])
                d[nm] = full.reshape(n_cores, *out_avals[i].shape)[c]
            res.append(d)
        return res

    return run


def kernel(**inputs):
    """Full (unsharded) inputs -> full (B, T, D) float32 output."""
    if "nc" not in _CACHE:
        _CACHE["nc"] = build_dit(n_cores=8)
        _CACHE["run"] = _runner(_CACHE["nc"], 8)
    in_maps = host_prep(**inputs)
    results = _CACHE["run"](in_maps)
    return host_post(results)
